# revision 26
# baseline (speedup 1.0000x reference)
"""Trainium2 Bass kernel for CrossAttention (B=4, T=2048, S=4096, D=256, H=8, Dh=32).

Sharding: 8 cores = 4 batches x 2 T-halves (each core owns TL=1024 query rows
of one batch, all heads, full context). No collectives; host concatenates.

Per-core dataflow (v2):
  Scores are fp16 with a uniform K=128 "zero-band" layout: kT4[j] [128, S]
  stacks 4 heads' 32-row k blocks; qTb[h] [128, TL] holds q_h in its band
  and ZEROS elsewhere, so every score matmul is a full [128,128]x[128,512]
  (measured 215.8ns back-to-back; K=32 matmuls run ~60% slower and PE
  geometry switches cost ~250ns, so the hot loop keeps one geometry).
  exp() is split: Act runs table exp -> fp8e4m3 values (with a 2^(C-G)
  fold so e4m3 never exceeds 240); DVE runs a one-pass Schraudolph affine
  (rounded, saturating fp32->uint16 conversion makes fp16 BITS directly;
  uint16 so negatives clamp to +0 instead of the fp16 NaN range).
  attn@v is mixed precision: Act-produced pairs go through fp8 DoubleRow
  matmuls (one instruction contracts TWO 128-row S-tiles; 2x fp16), DVE
  pairs through plain fp16 matmuls; both accumulate into the same PSUM
  with a [v(32) | 0(32) | ones | pad] stationary whose ones column yields
  the softmax denominator for free.
  Softmax division happens after attn@v: per-head reciprocal denominators
  (kept at partitions {0,32,64,96} - engine writes must be 32-aligned)
  are broadcast to the 128 hid rows by a selector matmul on the PE (no
  DMA round-trip), then outN = outU * rcp feeds the output projection.
"""

import sys

if "/opt/trn_rl_repo" not in sys.path:
    sys.path.insert(0, "/opt/trn_rl_repo")

from contextlib import ExitStack

import numpy as np

import concourse.bass as bass
import concourse.tile as tile
from concourse import bacc
from concourse import mybir
from concourse.bass_utils import run_bass_kernel_spmd

B, T, S, D, H, Dh = 4, 2048, 4096, 256, 8, 32
TL = T // 2  # 1024 query rows per core
NXT = TL // 128  # 8 x-tiles
NST = S // 128  # 32 s-tiles
SCALE = Dh**-0.5
FP = mybir.dt.float32
F16 = mybir.dt.float16
F8 = mybir.dt.float8e4
U16 = mybir.dt.uint16
DR = mybir.MatmulPerfMode.DoubleRow
LOG2E = float(np.log2(np.e))
LN2 = float(np.log(2.0))
G_OCT = 4.75  # global 2^-G scale: exp() stays <= 240 (e4m3-safe)
SCHRAUD_C = -0.3  # Schraudolph rounding bias (folded into BOTH exp paths)
VW = 66  # stationary cols per (head, s-tile): v(32) | zeros(32) | ones | pad

# exp engine per s-tile PAIR: Act (table exp -> fp8, ~1.0us/tile) or DVE
# (one-pass Schraudolph -> fp16 bits, ~1.2us/tile). 11 act / 5 dve per 16.
EXP_PAT = ["act", "dve", "act", "dve", "act", "dve", "act", "act",
           "dve", "act", "dve", "act", "dve", "act", "dve", "act"]
ACT_PAIRS = [p for p in range(NST // 2) if EXP_PAT[p % 16] == "act"]
DVE_PAIRS = [p for p in range(NST // 2) if EXP_PAT[p % 16] == "dve"]


def build_bass():
    nc = bacc.Bacc()
    ident_d = nc.declare_dram_parameter("ident", [128, 128], FP, isOutput=False)
    x_d = nc.declare_dram_parameter("x", [TL, D], FP, isOutput=False)
    ctx_d = nc.declare_dram_parameter("context", [S, D], FP, isOutput=False)
    wq_d = nc.declare_dram_parameter("w_q", [D, D], FP, isOutput=False)
    wkv_d = nc.declare_dram_parameter("w_kv", [D, 2 * D], FP, isOutput=False)
    wout_d = nc.declare_dram_parameter("w_out", [D, D], FP, isOutput=False)
    bout_d = nc.declare_dram_parameter("b_out", [1, D], FP, isOutput=False)
    out_d = nc.declare_dram_parameter("out", [TL, D], FP, isOutput=True)

    with tile.TileContext(nc) as tc, ExitStack() as ctx:
        consts = ctx.enter_context(tc.tile_pool(name="consts", bufs=1))
        persist = ctx.enter_context(tc.tile_pool(name="persist", bufs=1))
        psum = ctx.enter_context(tc.tile_pool(name="psum", bufs=3, space="PSUM"))
        accp = ctx.enter_context(tc.tile_pool(name="accp", bufs=1, space="PSUM"))

        identity = consts.tile([128, 128], FP, tag="identity", name="identity")
        idh = consts.tile([128, 128], F16, tag="idh", name="idh")

        wq16 = [persist.tile([128, D], F16, tag=f"wq{j}", name=f"wq{j}") for j in range(2)]
        wkv16 = [persist.tile([128, 2 * D], F16, tag=f"wkv{j}", name=f"wkv{j}") for j in range(2)]
        wout16 = [persist.tile([128, D], F16, tag=f"wo{j}", name=f"wo{j}") for j in range(2)]
        bias_b = persist.tile([128, D], FP, tag="bias_b", name="bias_b")

        xT = [persist.tile([128, TL], F16, tag=f"xT{j}", name=f"xT{j}") for j in range(2)]
        cT = [persist.tile([128, S], F16, tag=f"cT{j}", name=f"cT{j}") for j in range(2)]
        qTb = [persist.tile([128, TL], F16, tag=f"qTb{h}", name=f"qTb{h}") for h in range(H)]
        kT4 = [persist.tile([128, S], F16, tag=f"kT4{j}", name=f"kT4{j}") for j in range(2)]
        vcomb = persist.tile([128, NST, H, VW], F8, tag="vcomb", name="vcomb")
        DVE_STS = [2 * p + i for p in DVE_PAIRS for i in range(2)]
        DVE_ST_IDX = {st: k for k, st in enumerate(DVE_STS)}
        vc16 = persist.tile([128, len(DVE_STS), H, VW], F16, tag="vc16", name="vc16")
        outU = [persist.tile([128, TL], F16, tag=f"outU{j}", name=f"outU{j}") for j in range(2)]
        outN = [persist.tile([128, TL], F16, tag=f"outN{j}", name=f"outN{j}") for j in range(2)]
        ebias = persist.tile([128, 1], FP, tag="ebias", name="ebias")
        densX = [persist.tile([128, TL], FP, tag=f"densX{j}", name=f"densX{j}") for j in range(2)]
        rcpX = [persist.tile([128, TL], F16, tag=f"rcpX{j}", name=f"rcpX{j}") for j in range(2)]
        rcpF = [persist.tile([128, TL], FP, tag=f"rcpF{j}", name=f"rcpF{j}") for j in range(2)]
        sel128 = persist.tile([128, 128], F16, tag="sel128", name="sel128")

        rp = ctx.enter_context(tc.tile_pool(name="rings", bufs=3))
        early = tc.tile_pool(name="early", bufs=1)
        ep = early.__enter__()
        x_all = ep.tile([128, NXT, D], FP, tag="x_all", name="x_all")
        wstage = [ep.tile([128, 3 * D], FP, tag=f"wstage{j}", name=f"wstage{j}") for j in range(2)]
        wostage = [ep.tile([128, D], FP, tag=f"wos{j}", name=f"wos{j}") for j in range(2)]
        xh = ep.tile([128, NXT, D], F16, tag="xh", name="xh")

        # ---- Phase 0: DMAs. Queues are assigned round-robin in issue order
        # and run CONCURRENTLY (sharing HBM): put ident/x/weights as the
        # first transfer on each queue so they land early (~6us), with the
        # big context chunks queued BEHIND them.
        nc.sync.dma_start(out=identity, in_=ident_d[:, :])
        nc.sync.dma_start(out=x_all, in_=x_d.rearrange("(t p) d -> p t d", p=128))
        for j in range(2):
            nc.sync.dma_start(out=wstage[j][:, 0:D], in_=wq_d[128 * j : 128 * j + 128, :])
            nc.sync.dma_start(out=wstage[j][:, D : 3 * D], in_=wkv_d[128 * j : 128 * j + 128, :])
        nc.sync.dma_start(out=wostage[0], in_=wout_d[0:128, :])
        nc.sync.dma_start(out=wostage[1], in_=wout_d[128:256, :])
        ctx_r = ctx_d.rearrange("(t p) d -> p t d", p=128)
        nc.sync.dma_start(out=bias_b, in_=bout_d[0:1, :].partition_broadcast(128))
        c_tiles = []
        for cc in range(8):
            ctile = rp.tile([128, 4, D], FP, tag="cring", name=f"cring{cc}")
            nc.sync.dma_start(out=ctile, in_=ctx_r[:, 4 * cc : 4 * cc + 4, :])
            c_tiles.append(ctile)

        # DVE handles only the critical-path converts; every bulk memset
        # goes to the otherwise-idle gpsimd engine. Stationary cols 33-63/65
        # of vcomb/vc16 stay GARBAGE: they only feed acc rows that are never
        # read, so no zeroing is needed - just the ones column (denominator).
        nc.vector.tensor_copy(idh, identity)
        for j in range(2):
            nc.gpsimd.tensor_copy(wkv16[j], wstage[j][:, D : 3 * D])
        for h in range(H):
            nc.gpsimd.memset(qTb[h], 0.0)
        nc.gpsimd.memset(vcomb[:, :, :, 64:65], 1.0)
        nc.gpsimd.memset(vc16[:, :, :, 64:65], 1.0)
        nc.gpsimd.memset(ebias, (SCHRAUD_C - G_OCT) * LN2)
        nc.gpsimd.memset(sel128, 0.0)
        for u in range(4):
            nc.gpsimd.memset(sel128[32 * u : 32 * u + 1, 32 * u : 32 * u + 32], 1.0)
        for j in range(2):
            nc.gpsimd.memset(densX[j], 1.0)

        # ---- Phase 1+2: transposes + projections, chunk-chased ----
        nc.vector.tensor_copy(xh, x_all)
        for j in range(2):
            nc.vector.tensor_copy(wq16[j], wstage[j][:, 0:D])

        tcnt = [0]

        def transpose_batch(srct, lo, cnt, j, dst, dstoff=None):
            pt = psum.tile([128, 8, 128], F16, tag="sc", name="pt")
            for i in range(cnt):
                nc.tensor.transpose(pt[:, i : i + 1, :], srct[:, lo + i : lo + i + 1, 128 * j : 128 * j + 128], idh)
            k = tcnt[0] % 2
            tcnt[0] += 1
            if dstoff is None:
                dstoff = 128 * lo
            d = dst[:, dstoff : dstoff + cnt * 128]
            if k == 0:
                nc.vector.tensor_copy(d, pt[:, 0:cnt, :])
            else:
                nc.scalar.copy(d, pt[:, 0:cnt, :])

        for j in range(2):
            transpose_batch(xh, 0, 8, j, xT[j])
        # q projection (needs only x-side)
        for j in range(2):
            for nt in range(2):
                pq = psum.tile([128, 512], FP, tag="sc", name="pq")
                for kj in range(2):
                    nc.tensor.matmul(
                        pq,
                        lhsT=wq16[kj][:, 128 * j : 128 * j + 128],
                        rhs=xT[kj][:, 512 * nt : 512 * nt + 512],
                        start=(kj == 0),
                        stop=(kj == 1),
                    )
                for g in range(4):
                    dstq = qTb[4 * j + g][32 * g : 32 * g + 32, 512 * nt : 512 * nt + 512]
                    srcq = pq[32 * g : 32 * g + 32, :]
                    if g % 2 == 0:
                        nc.vector.tensor_copy(dstq, srcq)
                    else:
                        nc.scalar.copy(dstq, srcq)

        for j in range(2):
            nc.vector.tensor_copy(wout16[j], wostage[j])
        early.__exit__(None, None, None)
        dve16 = [pp % 16 for pp in DVE_PAIRS]
        atp = ctx.enter_context(tc.tile_pool(name="atp", bufs=11))
        tmps = ctx.enter_context(tc.tile_pool(name="tmps", bufs=15))
        fstage = ctx.enter_context(tc.tile_pool(name="fstage", bufs=4))

        A16 = SCALE * 1024.0 * LOG2E
        B16 = 1024.0 * (15.0 + SCHRAUD_C - G_OCT)
        head_state = {}

        def head_begin(h):
            st_ = {
                "acc": accp.tile([128, TL], FP, tag="acc", name="acc"),
                "at": {p: atp.tile([128, 2, TL], F8, tag="at", name="at") for p in ACT_PAIRS},
                "a16": {(p, i): tmps.tile([128, TL], U16, tag="t16", name="t16")
                        for p in DVE_PAIRS for i in range(2)},
                "dve_done": 0,
            }
            head_state[h] = st_
            return st_

        def emit_fp16_attnv_h(h, p, first):
            st_ = head_state[h]
            for i in range(2):
                st2 = DVE_ST_IDX[2 * p + i]
                for nt in range(2):
                    nc.tensor.matmul(
                        st_["acc"][0:VW, 512 * nt : 512 * nt + 512],
                        lhsT=vc16[:, st2 : st2 + 1, h : h + 1, 0:VW],
                        rhs=st_["a16"][(p, i)].bitcast(F16)[:, 512 * nt : 512 * nt + 512],
                        start=(first and i == 0),
                        stop=False,
                        skip_group_check=True,
                    )

        def emit_head_st(h, st):
            st_ = head_state[h]
            j = h // 4
            p, i = st // 2, st % 2
            sc = psum.tile([128, TL], FP, tag="sc", name="sc")
            for nt in range(2):
                nc.tensor.matmul(
                    sc[:, 512 * nt : 512 * nt + 512],
                    lhsT=kT4[j][:, 128 * st : 128 * st + 128],
                    rhs=qTb[h][:, 512 * nt : 512 * nt + 512],
                    start=True,
                    stop=True,
                    skip_group_check=True,
                )
            if p in st_["at"]:
                nc.scalar.activation(
                    st_["at"][p][:, i, :], sc,
                    mybir.ActivationFunctionType.Exp,
                    bias=ebias[:, 0:1], scale=SCALE,
                )
            else:
                nc.vector.tensor_scalar(
                    st_["a16"][(p, i)], sc, A16, B16,
                    mybir.AluOpType.mult, mybir.AluOpType.add,
                )
            while (st_["dve_done"] < len(DVE_PAIRS)
                   and 2 * DVE_PAIRS[st_["dve_done"]] + 1 <= st - 6):
                emit_fp16_attnv_h(h, DVE_PAIRS[st_["dve_done"]], first=(st_["dve_done"] == 0))
                st_["dve_done"] += 1

        def emit_head_finish(h):
            st_ = head_state[h]
            j, g = h // 4, h % 4
            while st_["dve_done"] < len(DVE_PAIRS):
                emit_fp16_attnv_h(h, DVE_PAIRS[st_["dve_done"]], first=(st_["dve_done"] == 0))
                st_["dve_done"] += 1
            acc = st_["acc"]
            for pi, p in enumerate(ACT_PAIRS):
                for nt in range(2):
                    nc.tensor.matmul(
                        acc[0:VW, 512 * nt : 512 * nt + 512],
                        lhsT=vcomb[:, 2 * p : 2 * p + 2, h : h + 1, 0:VW],
                        rhs=st_["at"][p][:, :, 512 * nt : 512 * nt + 512],
                        start=False,
                        stop=(pi == len(ACT_PAIRS) - 1),
                        perf_mode=DR,
                        skip_group_check=True,
                    )
            nc.vector.tensor_copy(outU[j][32 * g : 32 * g + 32, :], acc[0:32, :])
            nc.vector.tensor_copy(densX[j][32 * g : 32 * g + 32, :][0:1, :], acc[64:65, :])
            del head_state[h]

        def emit_head(h):
            head_begin(h)
            for st in range(NST):
                emit_head_st(h, st)
            emit_head_finish(h)

        head_begin(0)

        def chunk_hook(cc):
            for st in range(4 * cc, 4 * cc + 4):
                emit_head_st(0, st)
            if cc == 7:
                emit_head_finish(0)

        for cc in range(8):
            chct = rp.tile([128, 4, D], F16, tag="chcring", name=f"chc{cc}")
            nc.scalar.copy(chct, c_tiles[cc])
            for j in range(2):
                transpose_batch(chct, 0, 4, j, cT[j], 128 * 4 * cc)
            # k projection for this 512-col chunk
            for j in range(2):
                pk = psum.tile([128, 512], FP, tag="sc", name="pk")
                for kj in range(2):
                    nc.tensor.matmul(
                        pk,
                        lhsT=wkv16[kj][:, 128 * j : 128 * j + 128],
                        rhs=cT[kj][:, 512 * cc : 512 * cc + 512],
                        start=(kj == 0),
                        stop=(kj == 1),
                    )
                dstk = kT4[j][:, 512 * cc : 512 * cc + 512]
                if j % 2 == 0:
                    nc.vector.tensor_copy(dstk, pk)
                else:
                    nc.scalar.copy(dstk, pk)
            # v projection for this chunk's 4 s-tiles
            for st in range(4 * cc, 4 * cc + 4):
                pv = psum.tile([128, D], FP, tag="sc", name="pv")
                for kj in range(2):
                    nc.tensor.matmul(
                        pv,
                        lhsT=cT[kj][:, 128 * st : 128 * st + 128],
                        rhs=wkv16[kj][:, D : 2 * D],
                        start=(kj == 0),
                        stop=(kj == 1),
                    )
                if (st // 2) % 16 in dve16:
                    kk = DVE_ST_IDX[st]
                    vh_dst = vc16[:, kk : kk + 1, :, 0:32]
                else:
                    vh_dst = vcomb[:, st : st + 1, :, 0:32]
                if st % 2 == 0:
                    nc.vector.tensor_copy(vh_dst, pv)
                else:
                    nc.scalar.copy(vh_dst, pv)
            if chunk_hook is not None:
                chunk_hook(cc)

        # ---- Phase 3: attention ----
        # (head 0 was emitted inside the chunk loop via chunk_hook)
        for h in range(1, H):
            emit_head(h)

        # ---- Phase 4: normalize + output projection ----
        for j in range(2):
            nc.vector.reciprocal_approx_fast(rcpF[j], densX[j])
            nc.scalar.copy(rcpX[j], rcpF[j])
        for j in range(2):
            rb = accp.tile([128, TL], FP, tag="acc", name="rb")
            for nt in range(2):
                nc.tensor.matmul(
                    rb[:, 512 * nt : 512 * nt + 512],
                    lhsT=sel128,
                    rhs=rcpX[j][:, 512 * nt : 512 * nt + 512],
                    start=True,
                    stop=True,
                    skip_group_check=True,
                )
            nc.vector.tensor_mul(outN[j], outU[j], rb)
        for tt in range(TL // 128):
            fin = psum.tile([128, D], FP, tag="sc", name="fin")
            for j in range(2):
                nc.tensor.matmul(
                    fin,
                    lhsT=outN[j][:, 128 * tt : 128 * tt + 128],
                    rhs=wout16[j],
                    start=(j == 0),
                    stop=(j == 1),
                )
            outs = fstage.tile([128, D], FP, tag="outs", name="outs")
            nc.vector.tensor_add(outs, fin, bias_b)
            nc.sync.dma_start(out=out_d[128 * tt : 128 * tt + 128, :], in_=outs)

    nc.compile()
    return nc


_NC = None


def kernel(**inputs):
    global _NC
    x = np.ascontiguousarray(inputs["x"], dtype=np.float32)
    context = np.ascontiguousarray(inputs["context"], dtype=np.float32)
    w_q = np.ascontiguousarray(inputs["w_q"], dtype=np.float32)
    w_kv = np.ascontiguousarray(inputs["w_kv"], dtype=np.float32)
    w_out = np.ascontiguousarray(inputs["w_out"], dtype=np.float32)
    b_out = np.ascontiguousarray(inputs["b_out"], dtype=np.float32).reshape(1, D)

    if _NC is None:
        _NC = build_bass()
    nc = _NC

    in_maps = []
    for c in range(8):
        b, half = c // 2, c % 2
        in_maps.append(
            {
                "ident": np.eye(128, dtype=np.float32),
                "x": np.ascontiguousarray(x[b, TL * half : TL * half + TL, :]),
                "context": np.ascontiguousarray(context[b]),
                "w_q": w_q,
                "w_kv": w_kv,
                "w_out": w_out,
                "b_out": b_out,
            }
        )
    res = run_bass_kernel_spmd(nc, in_maps, core_ids=list(range(8)))
    out = np.empty((B, T, D), dtype=np.float32)
    for c in range(8):
        b, half = c // 2, c % 2
        out[b, TL * half : TL * half + TL, :] = res.results[c]["out"]
    return out


if __name__ == "__main__":
    rng = np.random.default_rng(0)
    ins = {
        "x": rng.standard_normal((B, T, D), dtype=np.float32),
        "context": rng.standard_normal((B, S, D), dtype=np.float32),
        "w_q": rng.standard_normal((D, D), dtype=np.float32) * D**-0.5,
        "w_kv": rng.standard_normal((D, 2 * D), dtype=np.float32) * D**-0.5,
        "w_out": rng.standard_normal((D, D), dtype=np.float32) * D**-0.5,
        "b_out": rng.standard_normal((D,), dtype=np.float32) * 0.01,
    }
    out = kernel(**ins)
    print(out.shape, out.dtype, np.abs(out).mean())


# revision 27
# speedup vs baseline: 1.0066x; 1.0066x over previous
"""Trainium2 Bass kernel for CrossAttention (B=4, T=2048, S=4096, D=256, H=8, Dh=32).

Sharding: 8 cores = 4 batches x 2 T-halves (each core owns TL=1024 query rows
of one batch, all heads, full context). No collectives; host concatenates.

Per-core dataflow (v2):
  Scores are fp16 with a uniform K=128 "zero-band" layout: kT4[j] [128, S]
  stacks 4 heads' 32-row k blocks; qTb[h] [128, TL] holds q_h in its band
  and ZEROS elsewhere, so every score matmul is a full [128,128]x[128,512]
  (measured 215.8ns back-to-back; K=32 matmuls run ~60% slower and PE
  geometry switches cost ~250ns, so the hot loop keeps one geometry).
  exp() is split: Act runs table exp -> fp8e4m3 values (with a 2^(C-G)
  fold so e4m3 never exceeds 240); DVE runs a one-pass Schraudolph affine
  (rounded, saturating fp32->uint16 conversion makes fp16 BITS directly;
  uint16 so negatives clamp to +0 instead of the fp16 NaN range).
  attn@v is mixed precision: Act-produced pairs go through fp8 DoubleRow
  matmuls (one instruction contracts TWO 128-row S-tiles; 2x fp16), DVE
  pairs through plain fp16 matmuls; both accumulate into the same PSUM
  with a [v(32) | 0(32) | ones | pad] stationary whose ones column yields
  the softmax denominator for free.
  Softmax division happens after attn@v: per-head reciprocal denominators
  (kept at partitions {0,32,64,96} - engine writes must be 32-aligned)
  are broadcast to the 128 hid rows by a selector matmul on the PE (no
  DMA round-trip), then outN = outU * rcp feeds the output projection.
"""

import sys

if "/opt/trn_rl_repo" not in sys.path:
    sys.path.insert(0, "/opt/trn_rl_repo")

from contextlib import ExitStack

import numpy as np

import concourse.bass as bass
import concourse.tile as tile
from concourse import bacc
from concourse import mybir
from concourse.bass_utils import run_bass_kernel_spmd

B, T, S, D, H, Dh = 4, 2048, 4096, 256, 8, 32
TL = T // 2  # 1024 query rows per core
NXT = TL // 128  # 8 x-tiles
NST = S // 128  # 32 s-tiles
SCALE = Dh**-0.5
FP = mybir.dt.float32
F16 = mybir.dt.float16
F8 = mybir.dt.float8e4
U16 = mybir.dt.uint16
DR = mybir.MatmulPerfMode.DoubleRow
LOG2E = float(np.log2(np.e))
LN2 = float(np.log(2.0))
G_OCT = 4.75  # global 2^-G scale: exp() stays <= 240 (e4m3-safe)
SCHRAUD_C = -0.3  # Schraudolph rounding bias (folded into BOTH exp paths)
VW = 66  # stationary cols per (head, s-tile): v(32) | zeros(32) | ones | pad

# exp engine per s-tile PAIR: Act (table exp -> fp8, ~1.0us/tile) or DVE
# (one-pass Schraudolph -> fp16 bits, ~1.2us/tile). 11 act / 5 dve per 16.
EXP_PAT = ["act", "dve", "act", "dve", "act", "act", "dve", "act",
           "dve", "act", "act", "dve", "act", "dve", "act", "act"]
ACT_PAIRS = [p for p in range(NST // 2) if EXP_PAT[p % 16] == "act"]
DVE_PAIRS = [p for p in range(NST // 2) if EXP_PAT[p % 16] == "dve"]


def build_bass():
    nc = bacc.Bacc()
    ident_d = nc.declare_dram_parameter("ident", [128, 128], FP, isOutput=False)
    x_d = nc.declare_dram_parameter("x", [TL, D], FP, isOutput=False)
    ctx_d = nc.declare_dram_parameter("context", [S, D], FP, isOutput=False)
    wq_d = nc.declare_dram_parameter("w_q", [D, D], FP, isOutput=False)
    wkv_d = nc.declare_dram_parameter("w_kv", [D, 2 * D], FP, isOutput=False)
    wout_d = nc.declare_dram_parameter("w_out", [D, D], FP, isOutput=False)
    bout_d = nc.declare_dram_parameter("b_out", [1, D], FP, isOutput=False)
    out_d = nc.declare_dram_parameter("out", [TL, D], FP, isOutput=True)

    with tile.TileContext(nc) as tc, ExitStack() as ctx:
        consts = ctx.enter_context(tc.tile_pool(name="consts", bufs=1))
        persist = ctx.enter_context(tc.tile_pool(name="persist", bufs=1))
        psum = ctx.enter_context(tc.tile_pool(name="psum", bufs=3, space="PSUM"))
        accp = ctx.enter_context(tc.tile_pool(name="accp", bufs=1, space="PSUM"))

        identity = consts.tile([128, 128], FP, tag="identity", name="identity")
        idh = consts.tile([128, 128], F16, tag="idh", name="idh")

        wq16 = [persist.tile([128, D], F16, tag=f"wq{j}", name=f"wq{j}") for j in range(2)]
        wkv16 = [persist.tile([128, 2 * D], F16, tag=f"wkv{j}", name=f"wkv{j}") for j in range(2)]
        wout16 = [persist.tile([128, D], F16, tag=f"wo{j}", name=f"wo{j}") for j in range(2)]
        bias_b = persist.tile([128, D], FP, tag="bias_b", name="bias_b")

        xT = [persist.tile([128, TL], F16, tag=f"xT{j}", name=f"xT{j}") for j in range(2)]
        cT = [persist.tile([128, S], F16, tag=f"cT{j}", name=f"cT{j}") for j in range(2)]
        qTb = [persist.tile([128, TL], F16, tag=f"qTb{h}", name=f"qTb{h}") for h in range(H)]
        kT4 = [persist.tile([128, S], F16, tag=f"kT4{j}", name=f"kT4{j}") for j in range(2)]
        vcomb = persist.tile([128, NST, H, VW], F8, tag="vcomb", name="vcomb")
        DVE_STS = [2 * p + i for p in DVE_PAIRS for i in range(2)]
        DVE_ST_IDX = {st: k for k, st in enumerate(DVE_STS)}
        vc16 = persist.tile([128, len(DVE_STS), H, VW], F16, tag="vc16", name="vc16")
        outU = [persist.tile([128, TL], F16, tag=f"outU{j}", name=f"outU{j}") for j in range(2)]
        outN = [persist.tile([128, TL], F16, tag=f"outN{j}", name=f"outN{j}") for j in range(2)]
        ebias = persist.tile([128, 1], FP, tag="ebias", name="ebias")
        densX = [persist.tile([128, TL], FP, tag=f"densX{j}", name=f"densX{j}") for j in range(2)]
        rcpX = [persist.tile([128, TL], F16, tag=f"rcpX{j}", name=f"rcpX{j}") for j in range(2)]
        rcpF = [persist.tile([128, TL], FP, tag=f"rcpF{j}", name=f"rcpF{j}") for j in range(2)]
        sel128 = persist.tile([128, 128], F16, tag="sel128", name="sel128")

        rp = ctx.enter_context(tc.tile_pool(name="rings", bufs=5))
        early = tc.tile_pool(name="early", bufs=1)
        ep = early.__enter__()
        x_all = ep.tile([128, NXT, D], FP, tag="x_all", name="x_all")
        wstage = [ep.tile([128, 3 * D], FP, tag=f"wstage{j}", name=f"wstage{j}") for j in range(2)]
        wostage = [ep.tile([128, D], FP, tag=f"wos{j}", name=f"wos{j}") for j in range(2)]
        xh = ep.tile([128, NXT, D], F16, tag="xh", name="xh")

        # ---- Phase 0: DMAs. Queues are assigned round-robin in issue order
        # and run CONCURRENTLY (sharing HBM): put ident/x/weights as the
        # first transfer on each queue so they land early (~6us), with the
        # big context chunks queued BEHIND them.
        nc.sync.dma_start(out=identity, in_=ident_d[:, :])
        nc.sync.dma_start(out=x_all, in_=x_d.rearrange("(t p) d -> p t d", p=128))
        for j in range(2):
            nc.sync.dma_start(out=wstage[j][:, 0:D], in_=wq_d[128 * j : 128 * j + 128, :])
            nc.sync.dma_start(out=wstage[j][:, D : 3 * D], in_=wkv_d[128 * j : 128 * j + 128, :])
        nc.sync.dma_start(out=wostage[0], in_=wout_d[0:128, :])
        nc.sync.dma_start(out=wostage[1], in_=wout_d[128:256, :])
        ctx_r = ctx_d.rearrange("(t p) d -> p t d", p=128)
        nc.sync.dma_start(out=bias_b, in_=bout_d[0:1, :].partition_broadcast(128))
        c_tiles = []
        for cc in range(8):
            ctile = rp.tile([128, 4, D], FP, tag="cring", name=f"cring{cc}")
            nc.sync.dma_start(out=ctile, in_=ctx_r[:, 4 * cc : 4 * cc + 4, :])
            c_tiles.append(ctile)

        # DVE handles only the critical-path converts; every bulk memset
        # goes to the otherwise-idle gpsimd engine. Stationary cols 33-63/65
        # of vcomb/vc16 stay GARBAGE: they only feed acc rows that are never
        # read, so no zeroing is needed - just the ones column (denominator).
        nc.vector.tensor_copy(idh, identity)
        for j in range(2):
            nc.gpsimd.tensor_copy(wkv16[j], wstage[j][:, D : 3 * D])
        for h in range(H):
            nc.gpsimd.memset(qTb[h], 0.0)
        nc.gpsimd.memset(vcomb[:, :, :, 64:65], 1.0)
        nc.gpsimd.memset(vc16[:, :, :, 64:65], 1.0)
        nc.gpsimd.memset(ebias, (SCHRAUD_C - G_OCT) * LN2)
        nc.gpsimd.memset(sel128, 0.0)
        for u in range(4):
            nc.gpsimd.memset(sel128[32 * u : 32 * u + 1, 32 * u : 32 * u + 32], 1.0)
        for j in range(2):
            nc.gpsimd.memset(densX[j], 1.0)

        # ---- Phase 1+2: transposes + projections, chunk-chased ----
        nc.vector.tensor_copy(xh, x_all)
        for j in range(2):
            nc.vector.tensor_copy(wq16[j], wstage[j][:, 0:D])

        tcnt = [0]

        def transpose_batch(srct, lo, cnt, j, dst, dstoff=None):
            pt = psum.tile([128, 8, 128], F16, tag="sc", name="pt")
            for i in range(cnt):
                nc.tensor.transpose(pt[:, i : i + 1, :], srct[:, lo + i : lo + i + 1, 128 * j : 128 * j + 128], idh)
            k = tcnt[0] % 2
            tcnt[0] += 1
            if dstoff is None:
                dstoff = 128 * lo
            d = dst[:, dstoff : dstoff + cnt * 128]
            if k == 0:
                nc.vector.tensor_copy(d, pt[:, 0:cnt, :])
            else:
                nc.scalar.copy(d, pt[:, 0:cnt, :])

        for j in range(2):
            transpose_batch(xh, 0, 8, j, xT[j])
        # q projection (needs only x-side)
        for j in range(2):
            for nt in range(2):
                pq = psum.tile([128, 512], FP, tag="sc", name="pq")
                for kj in range(2):
                    nc.tensor.matmul(
                        pq,
                        lhsT=wq16[kj][:, 128 * j : 128 * j + 128],
                        rhs=xT[kj][:, 512 * nt : 512 * nt + 512],
                        start=(kj == 0),
                        stop=(kj == 1),
                    )
                for g in range(4):
                    dstq = qTb[4 * j + g][32 * g : 32 * g + 32, 512 * nt : 512 * nt + 512]
                    srcq = pq[32 * g : 32 * g + 32, :]
                    if g % 2 == 0:
                        nc.vector.tensor_copy(dstq, srcq)
                    else:
                        nc.scalar.copy(dstq, srcq)

        for j in range(2):
            nc.vector.tensor_copy(wout16[j], wostage[j])
        early.__exit__(None, None, None)
        dve16 = [pp % 16 for pp in DVE_PAIRS]
        atp = ctx.enter_context(tc.tile_pool(name="atp", bufs=11))
        tmps = ctx.enter_context(tc.tile_pool(name="tmps", bufs=15))
        fstage = ctx.enter_context(tc.tile_pool(name="fstage", bufs=4))

        A16 = SCALE * 1024.0 * LOG2E
        B16 = 1024.0 * (15.0 + SCHRAUD_C - G_OCT)
        head_state = {}

        def head_begin(h):
            st_ = {
                "acc": accp.tile([128, TL], FP, tag="acc", name="acc"),
                "at": {p: atp.tile([128, 2, TL], F8, tag="at", name="at") for p in ACT_PAIRS},
                "a16": {(p, i): tmps.tile([128, TL], U16, tag="t16", name="t16")
                        for p in DVE_PAIRS for i in range(2)},
                "dve_done": 0,
            }
            head_state[h] = st_
            return st_

        def emit_fp16_attnv_h(h, p, first):
            st_ = head_state[h]
            for i in range(2):
                st2 = DVE_ST_IDX[2 * p + i]
                for nt in range(2):
                    nc.tensor.matmul(
                        st_["acc"][0:VW, 512 * nt : 512 * nt + 512],
                        lhsT=vc16[:, st2 : st2 + 1, h : h + 1, 0:VW],
                        rhs=st_["a16"][(p, i)].bitcast(F16)[:, 512 * nt : 512 * nt + 512],
                        start=(first and i == 0),
                        stop=False,
                        skip_group_check=True,
                    )

        def emit_head_st(h, st):
            st_ = head_state[h]
            j = h // 4
            p, i = st // 2, st % 2
            sc = psum.tile([128, TL], FP, tag="sc", name="sc")
            for nt in range(2):
                nc.tensor.matmul(
                    sc[:, 512 * nt : 512 * nt + 512],
                    lhsT=kT4[j][:, 128 * st : 128 * st + 128],
                    rhs=qTb[h][:, 512 * nt : 512 * nt + 512],
                    start=True,
                    stop=True,
                    skip_group_check=True,
                )
            if p in st_["at"]:
                nc.scalar.activation(
                    st_["at"][p][:, i, :], sc,
                    mybir.ActivationFunctionType.Exp,
                    bias=ebias[:, 0:1], scale=SCALE,
                )
            else:
                nc.vector.tensor_scalar(
                    st_["a16"][(p, i)], sc, A16, B16,
                    mybir.AluOpType.mult, mybir.AluOpType.add,
                )
            while (st_["dve_done"] < len(DVE_PAIRS)
                   and 2 * DVE_PAIRS[st_["dve_done"]] + 1 <= st - 6):
                emit_fp16_attnv_h(h, DVE_PAIRS[st_["dve_done"]], first=(st_["dve_done"] == 0))
                st_["dve_done"] += 1

        def emit_head_finish(h):
            st_ = head_state[h]
            j, g = h // 4, h % 4
            while st_["dve_done"] < len(DVE_PAIRS):
                emit_fp16_attnv_h(h, DVE_PAIRS[st_["dve_done"]], first=(st_["dve_done"] == 0))
                st_["dve_done"] += 1
            acc = st_["acc"]
            for pi, p in enumerate(ACT_PAIRS):
                for nt in range(2):
                    nc.tensor.matmul(
                        acc[0:VW, 512 * nt : 512 * nt + 512],
                        lhsT=vcomb[:, 2 * p : 2 * p + 2, h : h + 1, 0:VW],
                        rhs=st_["at"][p][:, :, 512 * nt : 512 * nt + 512],
                        start=False,
                        stop=(pi == len(ACT_PAIRS) - 1),
                        perf_mode=DR,
                        skip_group_check=True,
                    )
            nc.vector.tensor_copy(outU[j][32 * g : 32 * g + 32, :], acc[0:32, :])
            nc.vector.tensor_copy(densX[j][32 * g : 32 * g + 32, :][0:1, :], acc[64:65, :])
            del head_state[h]

        def emit_head(h):
            head_begin(h)
            for st in range(NST):
                emit_head_st(h, st)
            emit_head_finish(h)

        head_begin(0)

        def chunk_hook(cc):
            for st in range(4 * cc, 4 * cc + 4):
                emit_head_st(0, st)
            if cc == 7:
                emit_head_finish(0)

        for cc in range(8):
            chct = rp.tile([128, 4, D], F16, tag="chcring", name=f"chc{cc}")
            nc.scalar.copy(chct, c_tiles[cc])
            for j in range(2):
                transpose_batch(chct, 0, 4, j, cT[j], 128 * 4 * cc)
            # k projection for this 512-col chunk
            for j in range(2):
                pk = psum.tile([128, 512], FP, tag="sc", name="pk")
                for kj in range(2):
                    nc.tensor.matmul(
                        pk,
                        lhsT=wkv16[kj][:, 128 * j : 128 * j + 128],
                        rhs=cT[kj][:, 512 * cc : 512 * cc + 512],
                        start=(kj == 0),
                        stop=(kj == 1),
                    )
                dstk = kT4[j][:, 512 * cc : 512 * cc + 512]
                if j % 2 == 0:
                    nc.vector.tensor_copy(dstk, pk)
                else:
                    nc.scalar.copy(dstk, pk)
            # v projection for this chunk's 4 s-tiles
            for st in range(4 * cc, 4 * cc + 4):
                pv = psum.tile([128, D], FP, tag="sc", name="pv")
                for kj in range(2):
                    nc.tensor.matmul(
                        pv,
                        lhsT=cT[kj][:, 128 * st : 128 * st + 128],
                        rhs=wkv16[kj][:, D : 2 * D],
                        start=(kj == 0),
                        stop=(kj == 1),
                    )
                if (st // 2) % 16 in dve16:
                    kk = DVE_ST_IDX[st]
                    vh_dst = vc16[:, kk : kk + 1, :, 0:32]
                else:
                    vh_dst = vcomb[:, st : st + 1, :, 0:32]
                if st % 2 == 0:
                    nc.vector.tensor_copy(vh_dst, pv)
                else:
                    nc.scalar.copy(vh_dst, pv)
            if chunk_hook is not None:
                chunk_hook(cc)

        # ---- Phase 3: attention ----
        # (head 0 was emitted inside the chunk loop via chunk_hook)
        for h in range(1, H):
            emit_head(h)

        # ---- Phase 4: normalize + output projection ----
        for j in range(2):
            nc.vector.reciprocal_approx_fast(rcpF[j], densX[j])
            nc.scalar.copy(rcpX[j], rcpF[j])
        for j in range(2):
            rb = accp.tile([128, TL], FP, tag="acc", name="rb")
            for nt in range(2):
                nc.tensor.matmul(
                    rb[:, 512 * nt : 512 * nt + 512],
                    lhsT=sel128,
                    rhs=rcpX[j][:, 512 * nt : 512 * nt + 512],
                    start=True,
                    stop=True,
                    skip_group_check=True,
                )
            nc.vector.tensor_mul(outN[j], outU[j], rb)
        for tt in range(TL // 128):
            fin = psum.tile([128, D], FP, tag="sc", name="fin")
            for j in range(2):
                nc.tensor.matmul(
                    fin,
                    lhsT=outN[j][:, 128 * tt : 128 * tt + 128],
                    rhs=wout16[j],
                    start=(j == 0),
                    stop=(j == 1),
                )
            outs = fstage.tile([128, D], FP, tag="outs", name="outs")
            nc.vector.tensor_add(outs, fin, bias_b)
            nc.sync.dma_start(out=out_d[128 * tt : 128 * tt + 128, :], in_=outs)

    nc.compile()
    return nc


_NC = None


def kernel(**inputs):
    global _NC
    x = np.ascontiguousarray(inputs["x"], dtype=np.float32)
    context = np.ascontiguousarray(inputs["context"], dtype=np.float32)
    w_q = np.ascontiguousarray(inputs["w_q"], dtype=np.float32)
    w_kv = np.ascontiguousarray(inputs["w_kv"], dtype=np.float32)
    w_out = np.ascontiguousarray(inputs["w_out"], dtype=np.float32)
    b_out = np.ascontiguousarray(inputs["b_out"], dtype=np.float32).reshape(1, D)

    if _NC is None:
        _NC = build_bass()
    nc = _NC

    in_maps = []
    for c in range(8):
        b, half = c // 2, c % 2
        in_maps.append(
            {
                "ident": np.eye(128, dtype=np.float32),
                "x": np.ascontiguousarray(x[b, TL * half : TL * half + TL, :]),
                "context": np.ascontiguousarray(context[b]),
                "w_q": w_q,
                "w_kv": w_kv,
                "w_out": w_out,
                "b_out": b_out,
            }
        )
    res = run_bass_kernel_spmd(nc, in_maps, core_ids=list(range(8)))
    out = np.empty((B, T, D), dtype=np.float32)
    for c in range(8):
        b, half = c // 2, c % 2
        out[b, TL * half : TL * half + TL, :] = res.results[c]["out"]
    return out


if __name__ == "__main__":
    rng = np.random.default_rng(0)
    ins = {
        "x": rng.standard_normal((B, T, D), dtype=np.float32),
        "context": rng.standard_normal((B, S, D), dtype=np.float32),
        "w_q": rng.standard_normal((D, D), dtype=np.float32) * D**-0.5,
        "w_kv": rng.standard_normal((D, 2 * D), dtype=np.float32) * D**-0.5,
        "w_out": rng.standard_normal((D, D), dtype=np.float32) * D**-0.5,
        "b_out": rng.standard_normal((D,), dtype=np.float32) * 0.01,
    }
    out = kernel(**ins)
    print(out.shape, out.dtype, np.abs(out).mean())
